# revision 2
# baseline (speedup 1.0000x reference)
"""Block-diagonal 2x2 equalizer kernel for Trainium2 (8 NeuronCores), v4.

Per point (b, u, s, f) solves the 2x2 system M x = v by Cramer's rule:
    m_ij = h[b, pi[u], i, 0, 2u+j, s, f]   (only 1/4 of h is needed)
    det  = m00*m11 - m01*m10               (fp32 only: min |det| ~ 1.5e-4)
    x0   = (m11*v0 - m01*v1) / det         (numerators tolerate fp16)
    x1   = (m00*v1 - m10*v0) / det

Sharding: pure data parallel over batch, 2 batches per core on 8 cores.
Measured HW facts driving this structure (microbenched this session):
  - DVE TensorTensor: fp32 1x (0.96GHz), fp16 2x when all last dims are
    packed 2-byte.  STT/TensorScalarPtr is 1x always (no help).  GPSIMD
    concurrency HALVES DVE throughput -> gpsimd stays idle.
  - DVE has a hard ~7 cyc/pt floor: P(2) det(1) QQ(2) R(1) X(1) -> the
    stream is DVE-bound at ~13.1us + per-op overhead (~80ns/op).
  - Per-core DMA streams ~400GB/s aggregate on one HWDGE ring; input
    4.59MB ~ 11.5us < DVE busy, so DMA fully hides behind compute once
    the first chunk lands.
  - SP DMA trigger costs ~680ns of sequencer time; ~0.8us DGE fetch
    latency from trigger to first byte.
  - Fixed NEFF tail: all-engine barrier + 255 per-sem clears split
    across 5 engines (~7.5-9us) - identical for a trivial kernel, not
    kernel-controllable.

v4 vs v3.1: 6 size-graded chunks [128,256,448,448,320,192] instead of 8
equal ones (fewer DVE/ACT ops, small first chunk starts DVE ~2us
earlier, small last chunk shortens the tail chain), DVE order
P0 det0 | P_{k+1} det_{k+1} QQ_k R_k X_k | ... so cross-engine waits
always have >= 1 chunk of slack, per-chunk AB+V DMAs issued
back-to-back so the feed never starves DVE mid-stream.

Packing per chunk k of width W cols (col = one (b,u,s,f) point,
partition-major [128 x 1792] per core):
  AB = {m01|m11|m10|m00} fp32 (A|B), V = {v0|v1} fp16
  P  = A*B = {m01*m10 | m11*m00};  det = P[W:] - P[:W]
  ABf (ACT cvts, A halves swapped) = {m11f|m01f|m10f|m00f}
  QQ = ABf * {V|V}broadcast = {q0|q1|q3|q2}
  R  = {q0-q1 | q2-q3} (strided);  X = R * recip(det)_broadcast
"""

from contextlib import ExitStack

import numpy as np

import concourse.bass as bass
import concourse.mybir as mybir
from concourse.bass_utils import run_bass_kernel_spmd

# Problem shapes (hardcoded per contract)
B, U, A, NTX, T, S, F = 16, 4, 2, 1, 8, 14, 2048
SF = S * F               # 28672
NCORES = 8
BPC = B // NCORES        # 2 batches per core
PTS = BPC * U * SF       # 229376 points per core
COLS = PTS // 128        # 1792

CH = [128, 256, 448, 448, 320, 192]   # chunk widths (cols), sum = COLS
NCH = len(CH)
C0 = [sum(CH[:k]) for k in range(NCH)]          # col offsets
AB_OFF = []                                      # byte offsets of ab_k in dIn row
V_OFF = []
_off = 0
for _w in CH:
    AB_OFF.append(_off)
    _off += 16 * _w
    V_OFF.append(_off)
    _off += 4 * _w
TOTB = _off                                      # 35840
# output store groups (chunk index ranges)
STORES = [(0, 3), (3, 5), (5, 6)]

TRACE = False
LAST_RESULTS = None

f32 = mybir.dt.float32
f16 = mybir.dt.float16
u8 = mybir.dt.uint8


def _build_nc():
    nc = bass.Bass("TRN2")
    dIn = nc.dram_tensor("dIn", [128, TOTB], u8, kind="ExternalInput")
    xO = nc.dram_tensor("xO", [128, 2 * COLS], f16, kind="ExternalOutput")

    with ExitStack() as ctx:
        sb = lambda name, w, dt: ctx.enter_context(nc.sbuf_tensor(name, [128, w], dt))
        tIn = sb("tIn", TOTB, u8)
        tABf = [sb(f"tABf{k}", 4 * w, f16) for k, w in enumerate(CH)]
        tP = [sb(f"tP{k}", 2 * w, f32) for k, w in enumerate(CH)]
        tDet = [sb(f"tDet{k}", w, f32) for k, w in enumerate(CH)]
        tRd = [sb(f"tRd{k}", w, f16) for k, w in enumerate(CH)]
        tQ = [sb(f"tQ{k}", 4 * w, f16) for k, w in enumerate(CH)]
        tR = [sb(f"tR{k}", 2 * w, f16) for k, w in enumerate(CH)]
        tX = sb("tX", 2 * COLS, f16)
        tWarm = sb("tWarm", 1, f16)

        vA = [tIn[:, AB_OFF[k]:AB_OFF[k] + 8 * w].bitcast(f32) for k, w in enumerate(CH)]
        vB = [
            tIn[:, AB_OFF[k] + 8 * w:AB_OFF[k] + 16 * w].bitcast(f32)
            for k, w in enumerate(CH)
        ]
        vV = [tIn[:, V_OFF[k]:V_OFF[k] + 4 * w].bitcast(f16) for k, w in enumerate(CH)]

        abS = [ctx.enter_context(nc.semaphore(f"abS{k}")) for k in range(NCH)]
        vS = [ctx.enter_context(nc.semaphore(f"vS{k}")) for k in range(NCH)]
        dveS = ctx.enter_context(nc.semaphore("dveS"))
        actS = ctx.enter_context(nc.semaphore("actS"))
        outS = ctx.enter_context(nc.semaphore("outS"))

        # DVE op schedule: P0 det0, then for k: [P_{k+1} det_{k+1}] QQ_k R_k X_k
        sched = [("P", 0), ("D", 0)]
        for k in range(NCH):
            if k + 1 < NCH:
                sched += [("P", k + 1), ("D", k + 1)]
            sched += [("Q", k), ("R", k), ("X", k)]
        det_done = {}
        x_done = {}
        for i, (kind, k) in enumerate(sched):
            if kind == "D":
                det_done[k] = i + 1
            if kind == "X":
                x_done[k] = i + 1
        # ACT: warm, then per chunk 3 cvts + recip
        cvt_done = {k: 4 * k + 3 for k in range(NCH)}
        recip_done = {k: 4 * k + 4 for k in range(NCH)}

        with nc.Block(no_gpsimd_drain=True) as block:

            @block.scalar
            def _(scalar):
                # dummy activation: forces the one-time ACT_TABLE_LOAD during
                # the first DMA flight
                scalar.copy(tWarm[:], nc.const_aps.aps[(f32, 0.0)])
                for k, w in enumerate(CH):
                    scalar.wait_ge(abS[k], 16)
                    # ABf = {m11f|m01f|m10f|m00f}; A = {m01|m11} swapped halves
                    scalar.copy(tABf[k][:, :w], vA[k][:, w:]).then_inc(actS, 1)
                    scalar.copy(tABf[k][:, w:2 * w], vA[k][:, :w]).then_inc(actS, 1)
                    scalar.copy(tABf[k][:, 2 * w:], vB[k]).then_inc(actS, 1)
                    scalar.wait_ge(dveS, det_done[k])
                    scalar.add_instruction(
                        mybir.InstActivation(
                            name=nc.get_next_instruction_name(),
                            func=mybir.ActivationFunctionType.Reciprocal,
                            ins=[
                                scalar.lower_ap(tDet[k][:]),
                                mybir.ImmediateValue(dtype=f32, value=0.0),
                                mybir.ImmediateValue(dtype=f32, value=1.0),
                                mybir.ImmediateValue(dtype=f32, value=0.0),
                            ],
                            outs=[scalar.lower_ap(tRd[k][:])],
                        )
                    ).then_inc(actS, 1)

            @block.sync
            def _(sync):
                for k in range(NCH):
                    sync.dma_start(
                        out=tIn[:, AB_OFF[k]:V_OFF[k]],
                        in_=dIn[:, AB_OFF[k]:V_OFF[k]],
                    ).then_inc(abS[k], 16)
                    sync.dma_start(
                        out=tIn[:, V_OFF[k]:V_OFF[k] + 4 * CH[k]],
                        in_=dIn[:, V_OFF[k]:V_OFF[k] + 4 * CH[k]],
                    ).then_inc(vS[k], 16)
                for lo, hi in STORES:
                    sync.wait_ge(dveS, x_done[hi - 1])
                    sync.dma_start(
                        out=xO[:, 2 * C0[lo]:2 * (C0[hi - 1] + CH[hi - 1])],
                        in_=tX[:, 2 * C0[lo]:2 * (C0[hi - 1] + CH[hi - 1])],
                    ).then_inc(outS, 16)
                sync.wait_ge(outS, len(STORES) * 16)

            @block.vector
            def _(vector):
                for kind, k in sched:
                    w = CH[k]
                    if kind == "P":
                        vector.wait_ge(abS[k], 16)
                        vector.tensor_mul(tP[k][:], vA[k], vB[k]).then_inc(dveS, 1)
                    elif kind == "D":
                        vector.tensor_sub(
                            tDet[k][:], tP[k][:, w:], tP[k][:, :w]
                        ).then_inc(dveS, 1)
                    elif kind == "Q":
                        vector.wait_ge(actS, cvt_done[k])
                        vector.wait_ge(vS[k], 16)
                        qq = tQ[k][:].rearrange("p (a c) -> p a c", a=2, c=2 * w)
                        abf = tABf[k][:].rearrange("p (a c) -> p a c", a=2, c=2 * w)
                        vbc = vV[k].unsqueeze(1).broadcast_to((128, 2, 2 * w))
                        vector.tensor_mul(qq, abf, vbc).then_inc(dveS, 1)
                    elif kind == "R":
                        q4 = tQ[k][:].rearrange("p (a c) -> p a c", a=4, c=w)
                        rr = tR[k][:].rearrange("p (a c) -> p a c", a=2, c=w)
                        vector.tensor_sub(rr, q4[:, 0::3], q4[:, 1:3]).then_inc(
                            dveS, 1
                        )
                    elif kind == "X":
                        vector.wait_ge(actS, recip_done[k])
                        xx = tX[:, 2 * C0[k]:2 * (C0[k] + w)].rearrange(
                            "p (a c) -> p a c", a=2, c=w
                        )
                        rr = tR[k][:].rearrange("p (a c) -> p a c", a=2, c=w)
                        rdb = tRd[k][:].unsqueeze(1).broadcast_to((128, 2, w))
                        vector.tensor_mul(xx, rr, rdb).then_inc(dveS, 1)

    return nc


def make_in_maps(y, h, precoding_ind):
    """Host-side gather + byte-pack. Returns per-core input maps."""
    y = np.asarray(y)
    h = np.asarray(h)
    pi = np.asarray(precoding_ind).astype(np.int64)

    hg = h[:, pi[0]]                                     # [B, U, A, NTX, T, S, F]
    hsel = np.stack(
        [hg[:, u, :, 0, 2 * u:2 * u + 2] for u in range(U)], axis=1
    )                                                    # [B, U, A(i), 2(j), S, F]
    hsel = np.ascontiguousarray(hsel).astype(np.float32)
    yr = np.ascontiguousarray(y).astype(np.float32)      # [B, U, A, S, F]

    in_maps = []
    for c in range(NCORES):
        b0 = c * BPC
        hs = hsel[b0:b0 + BPC]
        ys = yr[b0:b0 + BPC]
        # planes as [128, COLS] (partition-major over flat [BPC,U,S,F])
        pl = lambda a: np.ascontiguousarray(a).reshape(128, COLS)
        m00 = pl(hs[:, :, 0, 0])
        m01 = pl(hs[:, :, 0, 1])
        m10 = pl(hs[:, :, 1, 0])
        m11 = pl(hs[:, :, 1, 1])
        v0 = pl(ys[:, :, 0]).astype(np.float16)
        v1 = pl(ys[:, :, 1]).astype(np.float16)
        dIn = np.empty((128, TOTB), np.uint8)
        for k, w in enumerate(CH):
            c0, c1 = C0[k], C0[k] + w
            ab = np.concatenate(
                [m01[:, c0:c1], m11[:, c0:c1], m10[:, c0:c1], m00[:, c0:c1]],
                axis=1,
            )                                            # [128, 4w] f32
            vv = np.concatenate([v0[:, c0:c1], v1[:, c0:c1]], axis=1)  # [128,2w] f16
            dIn[:, AB_OFF[k]:V_OFF[k]] = ab.view(np.uint8)
            dIn[:, V_OFF[k]:V_OFF[k] + 4 * w] = vv.view(np.uint8)
        in_maps.append({"dIn": dIn})
    return in_maps


def assemble_output(results):
    """Per-core xO [128, 2*COLS] f16 -> full [B, U, A, S, F] f32."""
    out = np.empty((B, U, A, S, F), np.float32)
    x0 = np.empty((128, COLS), np.float32)
    x1 = np.empty((128, COLS), np.float32)
    for c in range(NCORES):
        xo = np.asarray(results[c]["xO"]).astype(np.float32)
        for k, w in enumerate(CH):
            c0 = C0[k]
            x0[:, c0:c0 + w] = xo[:, 2 * c0:2 * c0 + w]
            x1[:, c0:c0 + w] = xo[:, 2 * c0 + w:2 * c0 + 2 * w]
        out[c * BPC:(c + 1) * BPC, :, 0] = x0.reshape(BPC, U, S, F)
        out[c * BPC:(c + 1) * BPC, :, 1] = x1.reshape(BPC, U, S, F)
    return out


def kernel(y, h, precoding_ind):
    global LAST_RESULTS
    in_maps = make_in_maps(y, h, precoding_ind)
    nc = _build_nc()
    res = run_bass_kernel_spmd(nc, in_maps, list(range(NCORES)), trace=TRACE)
    LAST_RESULTS = res
    return assemble_output(res.results)


# revision 7
# speedup vs baseline: 1.0195x; 1.0195x over previous
"""Block-diagonal 2x2 equalizer kernel for Trainium2 (8 NeuronCores), v4.

Per point (b, u, s, f) solves the 2x2 system M x = v by Cramer's rule:
    m_ij = h[b, pi[u], i, 0, 2u+j, s, f]   (only 1/4 of h is needed)
    det  = m00*m11 - m01*m10               (fp32 only: min |det| ~ 1.5e-4)
    x0   = (m11*v0 - m01*v1) / det         (numerators tolerate fp16)
    x1   = (m00*v1 - m10*v0) / det

Sharding: pure data parallel over batch, 2 batches per core on 8 cores.
Measured HW facts driving this structure (microbenched this session):
  - DVE TensorTensor: fp32 1x (0.96GHz), fp16 2x when all last dims are
    packed 2-byte.  STT/TensorScalarPtr is 1x always (no help).  GPSIMD
    concurrency HALVES DVE throughput -> gpsimd stays idle.
  - DVE has a hard ~7 cyc/pt floor: P(2) det(1) QQ(2) R(1) X(1) -> the
    stream is DVE-bound at ~13.1us + per-op overhead (~80ns/op).
  - Per-core DMA streams ~400GB/s aggregate on one HWDGE ring; input
    4.59MB ~ 11.5us < DVE busy, so DMA fully hides behind compute once
    the first chunk lands.
  - SP DMA trigger costs ~680ns of sequencer time; ~0.8us DGE fetch
    latency from trigger to first byte.
  - Fixed NEFF tail: all-engine barrier + 255 per-sem clears split
    across 5 engines (~7.5-9us) - identical for a trivial kernel, not
    kernel-controllable.

v4 vs v3.1: 6 size-graded chunks [128,256,448,448,320,192] instead of 8
equal ones (fewer DVE/ACT ops, small first chunk starts DVE ~2us
earlier, small last chunk shortens the tail chain), DVE order
P0 det0 | P_{k+1} det_{k+1} QQ_k R_k X_k | ... so cross-engine waits
always have >= 1 chunk of slack, per-chunk AB+V DMAs issued
back-to-back so the feed never starves DVE mid-stream.

Packing per chunk k of width W cols (col = one (b,u,s,f) point,
partition-major [128 x 1792] per core):
  AB = {m01|m11|m10|m00} fp32 (A|B), V = {v0|v1} fp16
  P  = A*B = {m01*m10 | m11*m00};  det = P[W:] - P[:W]
  ABf (ACT cvts, A halves swapped) = {m11f|m01f|m10f|m00f}
  QQ = ABf * {V|V}broadcast = {q0|q1|q3|q2}
  R  = {q0-q1 | q2-q3} (strided);  X = R * recip(det)_broadcast
"""

from contextlib import ExitStack

import numpy as np

import concourse.bass as bass
import concourse.mybir as mybir
from concourse.bass_utils import run_bass_kernel_spmd

# Problem shapes (hardcoded per contract)
B, U, A, NTX, T, S, F = 16, 4, 2, 1, 8, 14, 2048
SF = S * F               # 28672
NCORES = 8
BPC = B // NCORES        # 2 batches per core
PTS = BPC * U * SF       # 229376 points per core
COLS = PTS // 128        # 1792

CH = [128, 256, 448, 448, 320, 192]   # chunk widths (cols), sum = COLS
NCH = len(CH)
C0 = [sum(CH[:k]) for k in range(NCH)]          # col offsets
# SBUF byte offsets of ab_k / v_k regions inside tIn (packed back to back)
AB_OFF = []
V_OFF = []
_off = 0
for _w in CH:
    AB_OFF.append(_off)
    _off += 16 * _w
    V_OFF.append(_off)
    _off += 4 * _w
TOTB = _off                                      # 35840
# output store groups (chunk index ranges)
STORES = [(0, 3), (3, 5), (5, 6)]

TRACE = False
LAST_RESULTS = None

f32 = mybir.dt.float32
f16 = mybir.dt.float16
u8 = mybir.dt.uint8


def _build_nc():
    nc = bass.Bass("TRN2")
    # per-chunk dram tensors: each DMA reads a fully contiguous block
    # (a flat [128, TOTB] layout scatters rows -> measured ~240GB/s vs ~400)
    dAB = [
        nc.dram_tensor(f"dAB{k}", [128, 16 * w], u8, kind="ExternalInput")
        for k, w in enumerate(CH)
    ]
    dV = [
        nc.dram_tensor(f"dV{k}", [128, 4 * w], u8, kind="ExternalInput")
        for k, w in enumerate(CH)
    ]
    xO = nc.dram_tensor("xO", [128, 2 * COLS], f16, kind="ExternalOutput")

    with ExitStack() as ctx:
        sb = lambda name, w, dt: ctx.enter_context(nc.sbuf_tensor(name, [128, w], dt))
        tIn = sb("tIn", TOTB, u8)
        tABf = [sb(f"tABf{k}", 4 * w, f16) for k, w in enumerate(CH)]
        tP = [sb(f"tP{k}", 2 * w, f32) for k, w in enumerate(CH)]
        tDet = [sb(f"tDet{k}", w, f32) for k, w in enumerate(CH)]
        tRd = [sb(f"tRd{k}", w, f16) for k, w in enumerate(CH)]
        tQ = [sb(f"tQ{k}", 4 * w, f16) for k, w in enumerate(CH)]
        tR = [sb(f"tR{k}", 2 * w, f16) for k, w in enumerate(CH)]
        tX = sb("tX", 2 * COLS, f16)
        tWarm = sb("tWarm", 1, f16)

        vA = [tIn[:, AB_OFF[k]:AB_OFF[k] + 8 * w].bitcast(f32) for k, w in enumerate(CH)]
        vB = [
            tIn[:, AB_OFF[k] + 8 * w:AB_OFF[k] + 16 * w].bitcast(f32)
            for k, w in enumerate(CH)
        ]
        vV = [tIn[:, V_OFF[k]:V_OFF[k] + 4 * w].bitcast(f16) for k, w in enumerate(CH)]

        abS = [ctx.enter_context(nc.semaphore(f"abS{k}")) for k in range(NCH)]
        vS = [ctx.enter_context(nc.semaphore(f"vS{k}")) for k in range(NCH)]
        dveS = ctx.enter_context(nc.semaphore("dveS"))
        actS = ctx.enter_context(nc.semaphore("actS"))
        outS = ctx.enter_context(nc.semaphore("outS"))

        # DVE op schedule: P0 det0, then for k: [P_{k+1} det_{k+1}] QQ_k R_k X_k
        sched = [("P", 0), ("D", 0)]
        for k in range(NCH):
            if k + 1 < NCH:
                sched += [("P", k + 1), ("D", k + 1)]
            sched += [("Q", k), ("R", k), ("X", k)]
        det_done = {}
        x_done = {}
        for i, (kind, k) in enumerate(sched):
            if kind == "D":
                det_done[k] = i + 1
            if kind == "X":
                x_done[k] = i + 1
        # ACT: warm, then per chunk 3 cvts + recip
        cvt_done = {k: 4 * k + 3 for k in range(NCH)}
        recip_done = {k: 4 * k + 4 for k in range(NCH)}

        with nc.Block(no_gpsimd_drain=True) as block:

            @block.scalar
            def _(scalar):
                # V DMAs ride the Activation HWDGE ring: warms in parallel
                # with the SP ring and keeps all sync-ring bytes for AB
                scalar.dma_start(
                    out=tIn[:, V_OFF[0]:V_OFF[0] + 4 * CH[0]], in_=dV[0][:, :]
                ).then_inc(vS[0], 16)
                scalar.dma_start(
                    out=tIn[:, V_OFF[1]:V_OFF[1] + 4 * CH[1]], in_=dV[1][:, :]
                ).then_inc(vS[1], 16)
                # dummy activation: forces the one-time ACT_TABLE_LOAD during
                # the first DMA flight
                scalar.copy(tWarm[:], nc.const_aps.aps[(f32, 0.0)])
                for k, w in enumerate(CH):
                    if k + 2 < NCH:
                        j = k + 2
                        scalar.dma_start(
                            out=tIn[:, V_OFF[j]:V_OFF[j] + 4 * CH[j]],
                            in_=dV[j][:, :],
                        ).then_inc(vS[j], 16)
                    scalar.wait_ge(abS[k], 16)
                    # ABf = {m11f|m01f|m10f|m00f}; A = {m01|m11} swapped halves
                    scalar.copy(tABf[k][:, :w], vA[k][:, w:]).then_inc(actS, 1)
                    scalar.copy(tABf[k][:, w:2 * w], vA[k][:, :w]).then_inc(actS, 1)
                    scalar.copy(tABf[k][:, 2 * w:], vB[k]).then_inc(actS, 1)
                    scalar.wait_ge(dveS, det_done[k])
                    scalar.add_instruction(
                        mybir.InstActivation(
                            name=nc.get_next_instruction_name(),
                            func=mybir.ActivationFunctionType.Reciprocal,
                            ins=[
                                scalar.lower_ap(tDet[k][:]),
                                mybir.ImmediateValue(dtype=f32, value=0.0),
                                mybir.ImmediateValue(dtype=f32, value=1.0),
                                mybir.ImmediateValue(dtype=f32, value=0.0),
                            ],
                            outs=[scalar.lower_ap(tRd[k][:])],
                        )
                    ).then_inc(actS, 1)

            @block.sync
            def _(sync):
                for k in range(NCH):
                    sync.dma_start(
                        out=tIn[:, AB_OFF[k]:V_OFF[k]], in_=dAB[k][:, :]
                    ).then_inc(abS[k], 16)
                for lo, hi in STORES:
                    sync.wait_ge(dveS, x_done[hi - 1])
                    sync.dma_start(
                        out=xO[:, 2 * C0[lo]:2 * (C0[hi - 1] + CH[hi - 1])],
                        in_=tX[:, 2 * C0[lo]:2 * (C0[hi - 1] + CH[hi - 1])],
                    ).then_inc(outS, 16)
                sync.wait_ge(outS, len(STORES) * 16)

            @block.vector
            def _(vector):
                for kind, k in sched:
                    w = CH[k]
                    if kind == "P":
                        vector.wait_ge(abS[k], 16)
                        vector.tensor_mul(tP[k][:], vA[k], vB[k]).then_inc(dveS, 1)
                    elif kind == "D":
                        vector.tensor_sub(
                            tDet[k][:], tP[k][:, w:], tP[k][:, :w]
                        ).then_inc(dveS, 1)
                    elif kind == "Q":
                        vector.wait_ge(actS, cvt_done[k])
                        vector.wait_ge(vS[k], 16)
                        qq = tQ[k][:].rearrange("p (a c) -> p a c", a=2, c=2 * w)
                        abf = tABf[k][:].rearrange("p (a c) -> p a c", a=2, c=2 * w)
                        vbc = vV[k].unsqueeze(1).broadcast_to((128, 2, 2 * w))
                        vector.tensor_mul(qq, abf, vbc).then_inc(dveS, 1)
                    elif kind == "R":
                        q4 = tQ[k][:].rearrange("p (a c) -> p a c", a=4, c=w)
                        rr = tR[k][:].rearrange("p (a c) -> p a c", a=2, c=w)
                        vector.tensor_sub(rr, q4[:, 0::3], q4[:, 1:3]).then_inc(
                            dveS, 1
                        )
                    elif kind == "X":
                        vector.wait_ge(actS, recip_done[k])
                        xx = tX[:, 2 * C0[k]:2 * (C0[k] + w)].rearrange(
                            "p (a c) -> p a c", a=2, c=w
                        )
                        rr = tR[k][:].rearrange("p (a c) -> p a c", a=2, c=w)
                        rdb = tRd[k][:].unsqueeze(1).broadcast_to((128, 2, w))
                        vector.tensor_mul(xx, rr, rdb).then_inc(dveS, 1)

    return nc


def make_in_maps(y, h, precoding_ind):
    """Host-side gather + byte-pack. Returns per-core input maps."""
    y = np.asarray(y)
    h = np.asarray(h)
    pi = np.asarray(precoding_ind).astype(np.int64)

    hg = h[:, pi[0]]                                     # [B, U, A, NTX, T, S, F]
    hsel = np.stack(
        [hg[:, u, :, 0, 2 * u:2 * u + 2] for u in range(U)], axis=1
    )                                                    # [B, U, A(i), 2(j), S, F]
    hsel = np.ascontiguousarray(hsel).astype(np.float32)
    yr = np.ascontiguousarray(y).astype(np.float32)      # [B, U, A, S, F]

    in_maps = []
    for c in range(NCORES):
        b0 = c * BPC
        hs = hsel[b0:b0 + BPC]
        ys = yr[b0:b0 + BPC]
        # planes as [128, COLS] (partition-major over flat [BPC,U,S,F])
        pl = lambda a: np.ascontiguousarray(a).reshape(128, COLS)
        m00 = pl(hs[:, :, 0, 0])
        m01 = pl(hs[:, :, 0, 1])
        m10 = pl(hs[:, :, 1, 0])
        m11 = pl(hs[:, :, 1, 1])
        v0 = pl(ys[:, :, 0]).astype(np.float16)
        v1 = pl(ys[:, :, 1]).astype(np.float16)
        im = {}
        for k, w in enumerate(CH):
            c0, c1 = C0[k], C0[k] + w
            ab = np.concatenate(
                [m01[:, c0:c1], m11[:, c0:c1], m10[:, c0:c1], m00[:, c0:c1]],
                axis=1,
            )                                            # [128, 4w] f32
            vv = np.concatenate([v0[:, c0:c1], v1[:, c0:c1]], axis=1)  # [128,2w] f16
            im[f"dAB{k}"] = np.ascontiguousarray(ab).view(np.uint8)
            im[f"dV{k}"] = np.ascontiguousarray(vv).view(np.uint8)
        in_maps.append(im)
    return in_maps


def assemble_output(results):
    """Per-core xO [128, 2*COLS] f16 -> full [B, U, A, S, F] f32."""
    out = np.empty((B, U, A, S, F), np.float32)
    x0 = np.empty((128, COLS), np.float32)
    x1 = np.empty((128, COLS), np.float32)
    for c in range(NCORES):
        xo = np.asarray(results[c]["xO"]).astype(np.float32)
        for k, w in enumerate(CH):
            c0 = C0[k]
            x0[:, c0:c0 + w] = xo[:, 2 * c0:2 * c0 + w]
            x1[:, c0:c0 + w] = xo[:, 2 * c0 + w:2 * c0 + 2 * w]
        out[c * BPC:(c + 1) * BPC, :, 0] = x0.reshape(BPC, U, S, F)
        out[c * BPC:(c + 1) * BPC, :, 1] = x1.reshape(BPC, U, S, F)
    return out


def kernel(y, h, precoding_ind):
    global LAST_RESULTS
    in_maps = make_in_maps(y, h, precoding_ind)
    nc = _build_nc()
    res = run_bass_kernel_spmd(nc, in_maps, list(range(NCORES)), trace=TRACE)
    LAST_RESULTS = res
    return assemble_output(res.results)
